# revision 10
# baseline (speedup 1.0000x reference)
"""Trainium2 Bass kernel for nn_CNN_Casual (LeNet-ish CNN, B=8192). v2.

Pure data parallel over 8 NeuronCores: 1024 samples/core, 8 blocks of 128.

Key structure (vs baseline):
  conv1  : fp8-e4m3 DoubleRow matmuls, data-corrected: stationary holds
           (x_hi, x_lo) planes (x = x_hi + x_lo, both e4m3; exact to ~2^-8),
           moving holds the masked Toeplitz weights (e4m3, scale S1)
           duplicated into both pair slots.  2 windows share one 2-bank
           psum tile; 2 chunk matmuls per window at 0.5 cycles/column.
           conv1 bias rides the sacrificial lo-slot row 127 (corner pixel
           lo-bits dropped; ~0.3% of one tap).
  pool1  : per window-pair, either
             M_D: one fused 6D reduce_max (4:1) from psum (DVE only), or
             M_A: ACT relu-copy psum->fp16 + two packed-fp16 2x tensor_max
           chosen per-wpair to balance DVE vs ACT.
  T1     : PE transposes into [120,512] psum; ACT Relu evict -> x2cat fp16
           [121, 512] (row 120 = ones for the conv2 bias row).
  conv2  : fp16, single-output-row accumulation: 8 rows x 5 matmuls
           [121 x 160], bias via the ones row (b2 in W row 120 of ki=0).
  pool2  : per row-pair: fused 4D reduce_max (M_D) or ACT copy + TT (M_A).
  T2/fc1 : transpose -> f_all [81, 1024] (ones row 80 -> fc1 bias);
           fc1o = ACT Relu(psf1), fp16.
  fc2    : psf2 [128, 10]; DVE add of f32 (fc2_b - 10) -> t1_all.
  softmax: one batched epilogue per core: Exp(80) + 6D reduce_sum + Ln +
           8 scalar subs; single output DMA.  Act tables load twice total.

dtypes: fp8 only where corrected (conv1 data path); fp16 elsewhere on PE;
fp32 PSUM + f32 fc2 bias add keep log_softmax exact to ~1.6e-2 max rel.
"""

from contextlib import ExitStack

import numpy as np
import ml_dtypes

import concourse.mybir as mybir
import concourse.tile as tile
from concourse import bacc
from concourse.bass_utils import run_bass_kernel_spmd

F32 = mybir.dt.float32
FP16 = mybir.dt.float16
FP8 = mybir.dt.float8e4
AF = mybir.ActivationFunctionType
AX = mybir.AxisListType
DR = mybir.MatmulPerfMode.DoubleRow
E4 = ml_dtypes.float8_e4m3

N_CORES = 8
B_TOTAL = 8192
B_CORE = B_TOTAL // N_CORES  # 1024
S1 = 32.0

# pool mode tables, tuned against TimelineSim: 'A' = ACT-copy path,
# 'D' = DVE direct reduce.  MODE1: 6 window-pairs/block; MODE2: 4 row-pairs.
MODE1 = ["ADADAD", "ADAADA"]  # alternating per block parity
MODE2 = ["DDDD", "DDDD"]


# --------------------------------------------------------------------------
# Host-side weight preparation
# --------------------------------------------------------------------------
def _q8(a):
    return a.astype(E4)


def _prep_weights(mask_w, conv1_w, conv1_b, conv2_w, conv2_b, fc1_w, fc1_b,
                  fc2_w, fc2_b):
    f32 = np.float32
    sig = (1.0 / (1.0 + np.exp(-mask_w.astype(f32)))).astype(f32)  # [28,28]

    # conv1 Toeplitz per window t=(w,h): [128, 480], n = (tr,tc,u,o,m)
    # out row p = 4w + 2u + tr, col q = 12h + 2m + tc; k = r*16 + c
    # value = conv1_w[o,0,ki,kj] * sig[p+ki, q+kj], ki=r-dp, kj=c-ql
    w1m = np.zeros((128, 12 * 960), E4)
    oo = np.arange(10)
    for w in range(6):
        for h in range(2):
            t = w * 2 + h
            wt = np.zeros((128, 480), f32)
            for u in range(2):
                for tr in range(2):
                    dp = 2 * u + tr
                    for m in range(6):
                        for tc in range(2):
                            ql = 2 * m + tc
                            for ki in range(5):
                                r = dp + ki
                                for kj in range(5):
                                    c = ql + kj
                                    n = tr * 240 + tc * 120 + u * 60 + oo * 6 + m
                                    wt[r * 16 + c, n] = (
                                        conv1_w[:, 0, ki, kj]
                                        * sig[4 * w + r, 12 * h + c])
            wq = _q8(wt * S1)
            slot1 = wq.copy()
            # bias value depends only on o: col n -> o = (n % 60) // 6
            nidx = np.arange(480)
            bias_n = conv1_b.astype(f32)[(nidx % 60) // 6] * S1
            slot1[127, :] = _q8(bias_n)
            for c2 in range(2):
                base = t * 960 + c2 * 480
                w1m[:, base:base + 240] = wq[:, c2 * 240:(c2 + 1) * 240]
                w1m[:, base + 240:base + 480] = slot1[:, c2 * 240:
                                                      (c2 + 1) * 240]

    # conv2 single-row Toeplitz: per ki [121, 160], n2 = tc*80 + o*4 + s
    # (q2 = 2s + tc); row (c*12 + j) = conv2_w[o,c,ki,j-q2]/S1; row 120 =
    # b2[o] for ki==0.
    w2m = np.zeros((121, 800), np.float16)
    for ki in range(5):
        blk = np.zeros((121, 160), f32)
        for c in range(10):
            for j in range(12):
                for o in range(20):
                    for s in range(4):
                        for tc in range(2):
                            q2 = 2 * s + tc
                            kj = j - q2
                            if 0 <= kj < 5:
                                blk[c * 12 + j, tc * 80 + o * 4 + s] = \
                                    conv2_w[o, c, ki, kj] / S1
        if ki == 0:
            o_of = (np.arange(160) % 80) // 4
            blk[120, :] = conv2_b.astype(f32)[o_of]
        w2m[:, ki * 160:(ki + 1) * 160] = blk.astype(np.float16)

    # fc1 weights per pooled-row group p': [81, 200]; row 80 = fc1_b (g0)
    fc1w4 = fc1_w.reshape(50, 20, 4, 4)  # [m, o2, p', s2]
    wfc1 = np.zeros((81, 200), np.float16)
    for p in range(4):
        wfc1[0:80, p * 50:(p + 1) * 50] = \
            fc1w4[:, :, p, :].reshape(50, 80).T.astype(np.float16)
    wfc1[80, 0:50] = fc1_b.astype(np.float16)

    # wfcb fp16 [81, 210]: fc2_w.T | wfc1
    wfcb = np.zeros((81, 210), np.float16)
    wfcb[0:50, 0:10] = fc2_w.T.astype(np.float16)
    wfcb[:, 10:210] = wfc1

    # cst f32 [128, 20]: doubled (fc2_b - 10)
    cst = np.tile(np.concatenate([fc2_b.astype(f32) - 10.0] * 2)
                  .reshape(1, 20), (128, 1)).astype(f32)

    idb = np.eye(128).astype(np.float16)
    return dict(w1m=w1m, w2m=w2m, wfcb=wfcb, cst=cst, idb=idb)


# --------------------------------------------------------------------------
# Device program
# --------------------------------------------------------------------------
def _build(b_core):
    assert b_core % 256 == 0
    n_pair = b_core // 256

    nc = bacc.Bacc("TRN2", target_bir_lowering=False, debug=False,
                   num_devices=N_CORES)

    xw_d = nc.dram_tensor("xw", [12, 128, 2 * b_core], FP8,
                          kind="ExternalInput").ap()
    w1m_d = nc.dram_tensor("w1m", [128, 11520], FP8,
                           kind="ExternalInput").ap()
    w2m_d = nc.dram_tensor("w2m", [121, 800], FP16, kind="ExternalInput").ap()
    wfcb_d = nc.dram_tensor("wfcb", [81, 210], FP16,
                            kind="ExternalInput").ap()
    cst_d = nc.dram_tensor("cst", [128, 20], F32, kind="ExternalInput").ap()
    idb_d = nc.dram_tensor("idb", [128, 128], FP16, kind="ExternalInput").ap()
    y = nc.dram_tensor("y", [b_core, 10], F32, kind="ExternalOutput").ap()

    MAX, ADD = mybir.AluOpType.max, mybir.AluOpType.add

    with tile.TileContext(nc) as tc, ExitStack() as ctx:
        consts = ctx.enter_context(tc.tile_pool(name="consts", bufs=1))
        identb = consts.tile([128, 128], FP16)
        nc.sync.dma_start(identb[:], idb_d)
        w1m_sb = consts.tile([128, 11520], FP8)
        w2m_sb = consts.tile([121, 800], FP16)
        wfcb_sb = consts.tile([81, 210], FP16)
        cst_sb = consts.tile([128, 20], F32)

        wfc2_sb = wfcb_sb[0:50, 0:10]
        wfc1_sb = wfcb_sb[:, 10:210]
        t1_all = consts.tile([128, 10 * 8], F32)

        xw_pool = ctx.enter_context(tc.tile_pool(name="xw", bufs=3))
        ps1_pool = ctx.enter_context(tc.tile_pool(name="ps1", bufs=2,
                                                  space="PSUM"))
        tmp_pool = ctx.enter_context(tc.tile_pool(name="tmpb", bufs=4))
        prp_pool = ctx.enter_context(tc.tile_pool(name="prp", bufs=8))
        tpw_pool = ctx.enter_context(tc.tile_pool(name="tpw", bufs=2,
                                                  space="PSUM"))
        x2_pool = ctx.enter_context(tc.tile_pool(name="x2", bufs=6))
        ps2_pool = ctx.enter_context(tc.tile_pool(name="ps2", bufs=2,
                                                  space="PSUM"))
        p2_pool = ctx.enter_context(tc.tile_pool(name="p2", bufs=5))
        f_pool = ctx.enter_context(tc.tile_pool(name="fp", bufs=2))
        fc1o_pool = ctx.enter_context(tc.tile_pool(name="fc1o", bufs=2))
        sm_pool = ctx.enter_context(tc.tile_pool(name="sm", bufs=1))

        n_blk = 2 * n_pair
        xw_tiles = {}
        state = {}      # blk -> (prp_t list, xwcat ref)
        f_cur = [None]  # f_all tile for the current pair

        def fetch_pair(p):
            if p >= n_pair or p in xw_tiles:
                return
            xwcat = xw_pool.tile([128, 12 * 512], FP8, name="xwcat", tag="xw")
            deng = nc.sync if p % 2 == 0 else nc.scalar
            deng.dma_start(
                xwcat.rearrange("p (t c) -> p t c", t=12),
                xw_d[:, :, p * 512:(p + 1) * 512]
                .rearrange("t p c -> p t c"))
            xw_tiles[p] = xwcat

        def stage_a(blk):
            """conv1 (fp8 DR) + pool1 per window pair."""
            pair, half = blk // 2, blk % 2
            xwcat = xw_tiles[pair]
            mode1 = MODE1[blk % 2]
            prp_t = []
            for wp in range(6):
                pst = ps1_pool.tile([128, 1024], F32, name="pst", tag="ps1")
                for wi in range(2):
                    t = wp * 2 + wi
                    if blk == 0 and t >= 4:
                        eng = nc.sync if t % 2 == 0 else nc.scalar
                        eng.dma_start(
                            w1m_sb[:, t * 960:(t + 1) * 960],
                            w1m_d[:, t * 960:(t + 1) * 960])
                    lhsT = xwcat[:, t * 512 + half * 256:
                                 t * 512 + half * 256 + 256] \
                        .rearrange("p (two m) -> p two m", two=2)
                    for c2 in range(2):
                        nc.tensor.matmul(
                            pst[:, wi * 512 + c2 * 240:
                                wi * 512 + c2 * 240 + 240],
                            lhsT,
                            w1m_sb[:, t * 960 + c2 * 480:
                                   t * 960 + (c2 + 1) * 480]
                            .rearrange("p (two f) -> p two f", two=2),
                            start=True, stop=True, perf_mode=DR)
                # pooling: psum n-order per window = (tr, tc, u, o, m),
                # windows at 512-elem offsets (480 used, 32 pad)
                prp = prp_pool.tile([128, 240], FP16, name="prp", tag="prp")
                prp_t.append(prp)
                # prp col order (u, o, h, m): u:120, o:12, h:6, m:1
                prp_v = prp.rearrange("p (u o h m) -> p h (u o) m",
                                      u=2, o=10, h=2)
                pwin = pst.rearrange("p (h z) -> p h z", h=2)[:, :, 0:480]
                if mode1[wp] == "D":
                    src6 = pwin.rearrange(
                        "p h (tr tc uo m) -> p h uo m tr tc",
                        tr=2, tc=2, uo=20)
                    nc.vector.reduce_max(prp_v, src6, axis=AX.XY)
                else:
                    tmp = tmp_pool.tile([128, 960], FP16, name="tmp",
                                        tag="tmp")
                    nc.scalar.activation(
                        tmp.rearrange("p (h f) -> p h f", h=2), pwin,
                        AF.Relu)
                    tv = tmp.rearrange("p (h tr f) -> p h tr f", h=2, tr=2)
                    rm = tmp_pool.tile([128, 480], FP16, name="rm", tag="rm")
                    rmv = rm.rearrange("p (h f) -> p h f", h=2)
                    nc.vector.tensor_max(rmv, tv[:, :, 0], tv[:, :, 1])
                    rv5 = rm.rearrange("p (h tc uo m) -> p h tc uo m",
                                       h=2, tc=2, uo=20)
                    nc.vector.tensor_max(prp_v, rv5[:, :, 0], rv5[:, :, 1])
            state[blk] = prp_t
            if half == 1:
                fetch_pair(pair + 1)
            if blk == 0:
                nc.sync.dma_start(w2m_sb[:], w2m_d)
                nc.scalar.dma_start(wfcb_sb[:], wfcb_d)
                nc.scalar.dma_start(cst_sb[:], cst_d)

        def stage_b(blk):
            """T1 + evict -> conv2 + pool2 -> T2 -> f_all; fc on odd blk."""
            pair, half = blk // 2, blk % 2
            prp_t = state.pop(blk)
            mode2 = MODE2[blk % 2]
            if half == 0:
                f_all = f_pool.tile([81, 1024], FP16, name="f_all",
                                    tag="f_all")
                nc.gpsimd.memset(f_all[80:81, :], 1.0)
                f_cur[0] = f_all
            f_all = f_cur[0]

            # ---- T1 transposes + relu evict -> x2cat ----
            x2cat = []
            for ww in range(3):
                tpw = tpw_pool.tile([120, 512], FP16, name="tpw", tag="tpw")
                for q in range(4):
                    r = ww * 4 + q          # pooled row 0..11
                    prp = prp_t[r // 2]
                    u = r % 2
                    nc.tensor.transpose(
                        tpw[:, q * 128:(q + 1) * 128],
                        prp[:, u * 120:u * 120 + 120], identb[:])
                x2c = x2_pool.tile([121, 512], FP16, name="x2c", tag="x2c")
                nc.gpsimd.memset(x2c[120:121, :], 1.0)
                nc.scalar.activation(x2c[0:120, :], tpw[:], AF.Relu)
                x2cat.append(x2c)

            # ---- conv2 (fp16 single-row) + pool2 + T2 ----
            tp2w = tpw_pool.tile([80, 512], FP16, name="tp2w", tag="tpw")
            for g in range(4):
                ps2 = ps2_pool.tile([128, 320], F32, name="ps2", tag="ps2")
                for sub in range(2):
                    p2r = g * 2 + sub
                    for ki in range(5):
                        i = p2r + ki
                        nc.tensor.matmul(
                            ps2[:, sub * 160:sub * 160 + 160],
                            x2cat[i // 4][0:121,
                                          (i % 4) * 128:(i % 4 + 1) * 128],
                            w2m_sb[:, ki * 160:(ki + 1) * 160],
                            start=(ki == 0), stop=(ki == 4))
                # pool2: region layout n2 = (tc, o, s); rows = pl
                p2t = p2_pool.tile([128, 80], FP16, name="p2t", tag="p2t")
                if mode2[g] == "D":
                    src = ps2.rearrange("p (pl tc os) -> p os pl tc",
                                        pl=2, tc=2)
                    nc.vector.reduce_max(p2t[:], src, axis=AX.XY)
                else:
                    tmp2 = tmp_pool.tile([128, 320], FP16, name="tmp2",
                                         tag="tmp2")
                    nc.scalar.activation(tmp2[:], ps2[:], AF.Relu)
                    t2v = tmp2.rearrange("p (pl f) -> p pl f", pl=2)
                    rm2 = tmp_pool.tile([128, 160], FP16, name="rm2",
                                        tag="rm2")
                    nc.vector.tensor_max(rm2[:], t2v[:, 0], t2v[:, 1])
                    r2v = rm2.rearrange("p (tc f) -> p tc f", tc=2)
                    nc.vector.tensor_max(p2t[:], r2v[:, 0], r2v[:, 1])
                nc.tensor.transpose(tp2w[:, g * 128:(g + 1) * 128],
                                    p2t[:], identb[:])
            nc.scalar.activation(f_all[0:80, half * 512:half * 512 + 512],
                                 tp2w[:], AF.Relu)
            if half == 0:
                return

            # ---- fc1 over the 256-sample pair ----
            psf1 = ps2_pool.tile([50, 256], F32, name="psf1", tag="ps2")
            for g in range(4):
                rows = 81 if g == 0 else 80
                fvg = f_all[0:rows, :].rearrange("p (h g n) -> p g h n",
                                                 h=2, g=4)[:, g]
                nc.tensor.matmul(psf1[:], wfc1_sb[0:rows, g * 50:g * 50 + 50],
                                 fvg, start=(g == 0), stop=(g == 3))
            fc1o = fc1o_pool.tile([50, 256], FP16, name="fc1o", tag="fc1o")
            nc.scalar.activation(fc1o[:], psf1[:], AF.Relu)
            # ---- fc2 ----
            psf2 = ps2_pool.tile([128, 20], F32, name="psf2", tag="ps2")
            for h2 in range(2):
                nc.tensor.matmul(psf2[:, h2 * 10:h2 * 10 + 10],
                                 fc1o[:, h2 * 128:h2 * 128 + 128],
                                 wfc2_sb[:], start=True, stop=True)
            nc.vector.tensor_add(t1_all[:, pair * 20:pair * 20 + 20],
                                 psf2[:], cst_sb[:])
            # per-pair Exp + sums keep the epilogue tail short (Exp is in
            # the same act-func set as Relu, so no extra table loads)
            nc.scalar.activation(e_all[:, pair * 20:pair * 20 + 20],
                                 t1_all[:, pair * 20:pair * 20 + 20], AF.Exp)
            nc.vector.reduce_sum(
                se[:, pair * 2:pair * 2 + 2],
                e_all[:, pair * 20:pair * 20 + 20]
                .rearrange("p (b t) -> p b t", t=10), axis=AX.X)

        # software-pipelined emission: stage_b(blk-1) before stage_a(blk)
        for t in range(4):
            eng = nc.sync if t % 2 == 0 else nc.scalar
            eng.dma_start(w1m_sb[:, t * 960:(t + 1) * 960],
                          w1m_d[:, t * 960:(t + 1) * 960])
        fetch_pair(0)
        e_all = sm_pool.tile([128, 80], F32, name="e_all", tag="e_all")
        se = sm_pool.tile([128, 8], F32, name="se", tag="se")
        for it in range(n_blk + 1):
            if it >= 1:
                stage_b(it - 1)
            if it < n_blk:
                stage_a(it)

        # ---- log_softmax tail: one Ln + subs ----
        ls = sm_pool.tile([128, 8], F32, name="ls", tag="ls")
        nc.scalar.activation(ls[:], se[:], AF.Ln)
        yo = sm_pool.tile([128, 80], F32, name="yo", tag="yo")
        for b in range(8):
            nc.vector.tensor_scalar_sub(yo[:, b * 10:b * 10 + 10],
                                        t1_all[:, b * 10:b * 10 + 10],
                                        ls[:, b:b + 1])
        nc.scalar.dma_start(
            y.rearrange("(blk p) c -> p blk c", p=128),
            yo.rearrange("p (blk c) -> p blk c", c=10))

    nc.compile()
    return nc


_PROGRAM_CACHE = {}


def _get_program(b_core):
    if b_core not in _PROGRAM_CACHE:
        _PROGRAM_CACHE[b_core] = _build(b_core)
    return _PROGRAM_CACHE[b_core]


def make_in_maps(x, weights, b_core=B_CORE, n_cores=N_CORES):
    """Shard x over cores; replicate the (rearranged) parameters."""
    f32 = np.float32
    xr = np.asarray(x, dtype=f32).reshape(-1, 28, 28)
    in_maps = []
    for cidx in range(n_cores):
        xc = xr[cidx * b_core:(cidx + 1) * b_core]  # [b_core, 28, 28]
        xwin = np.zeros((12, 128, 2 * b_core), E4)
        for w in range(6):
            for h in range(2):
                t = w * 2 + h
                win = xc[:, 4 * w:4 * w + 8, 12 * h:12 * h + 16] \
                    .reshape(b_core, 128).astype(f32)
                hi = win.astype(E4)
                lo = (win - hi.astype(f32)).astype(E4)
                lo_f = lo.astype(f32)
                lo_f[:, 127] = 1.0
                lo = lo_f.astype(E4)
                hiT = hi.astype(f32).T.astype(E4)   # [128, b_core]
                loT = lo.astype(f32).T.astype(E4)
                for blk in range(b_core // 128):
                    p, hb = blk // 2, blk % 2
                    base = p * 512 + hb * 256
                    xwin[t, :, base:base + 128] = \
                        hiT[:, blk * 128:(blk + 1) * 128]
                    xwin[t, :, base + 128:base + 256] = \
                        loT[:, blk * 128:(blk + 1) * 128]
        m = {"xw": xwin}
        m.update(weights)
        in_maps.append(m)
    return in_maps


def kernel(**inputs):
    x = np.asarray(inputs["x"], dtype=np.float32)
    weights = _prep_weights(
        np.asarray(inputs["mask_w"], np.float32),
        np.asarray(inputs["conv1_w"], np.float32),
        np.asarray(inputs["conv1_b"], np.float32),
        np.asarray(inputs["conv2_w"], np.float32),
        np.asarray(inputs["conv2_b"], np.float32),
        np.asarray(inputs["fc1_w"], np.float32),
        np.asarray(inputs["fc1_b"], np.float32),
        np.asarray(inputs["fc2_w"], np.float32),
        np.asarray(inputs["fc2_b"], np.float32),
    )
    nc = _get_program(B_CORE)
    in_maps = make_in_maps(x, weights)
    res = run_bass_kernel_spmd(nc, in_maps, list(range(N_CORES)))
    out = np.concatenate([res.results[c]["y"] for c in range(N_CORES)],
                         axis=0)
    return np.ascontiguousarray(out.astype(np.float32))


if __name__ == "__main__":
    rng = np.random.default_rng(0)
    ins = {
        "x": rng.standard_normal((B_TOTAL, 1, 28, 28), dtype=np.float32),
        "mask_w": rng.standard_normal((28, 28), dtype=np.float32) * 0.1,
        "conv1_w": rng.standard_normal((10, 1, 5, 5), dtype=np.float32) * 0.2,
        "conv1_b": rng.standard_normal((10,), dtype=np.float32) * 0.1,
        "conv2_w": rng.standard_normal((20, 10, 5, 5),
                                       dtype=np.float32) * 0.06,
        "conv2_b": rng.standard_normal((20,), dtype=np.float32) * 0.1,
        "fc1_w": rng.standard_normal((50, 320), dtype=np.float32) * 0.05,
        "fc1_b": rng.standard_normal((50,), dtype=np.float32) * 0.1,
        "fc2_w": rng.standard_normal((10, 50), dtype=np.float32) * 0.14,
        "fc2_b": rng.standard_normal((10,), dtype=np.float32) * 0.1,
    }
    out = kernel(**ins)
    print(out.shape, out.dtype, out[:2])


# revision 13
# speedup vs baseline: 1.0143x; 1.0143x over previous
"""Trainium2 Bass kernel for nn_CNN_Casual (LeNet-ish CNN, B=8192). v2.

Pure data parallel over 8 NeuronCores: 1024 samples/core, 8 blocks of 128.

Key structure (vs baseline):
  conv1  : fp8-e4m3 DoubleRow matmuls, data-corrected: stationary holds
           (x_hi, x_lo) planes (x = x_hi + x_lo, both e4m3; exact to ~2^-8),
           moving holds the masked Toeplitz weights (e4m3, scale S1)
           duplicated into both pair slots.  2 windows share one 2-bank
           psum tile; 2 chunk matmuls per window at 0.5 cycles/column.
           conv1 bias rides the sacrificial lo-slot row 127 (corner pixel
           lo-bits dropped; ~0.3% of one tap).
  pool1  : per window-pair, either
             M_D: one fused 6D reduce_max (4:1) from psum (DVE only), or
             M_A: ACT relu-copy psum->fp16 + two packed-fp16 2x tensor_max
           chosen per-wpair to balance DVE vs ACT.
  T1     : PE transposes into [120,512] psum; ACT Relu evict -> x2cat fp16
           [121, 512] (row 120 = ones for the conv2 bias row).
  conv2  : fp16, single-output-row accumulation: 8 rows x 5 matmuls
           [121 x 160], bias via the ones row (b2 in W row 120 of ki=0).
  pool2  : per row-pair: fused 4D reduce_max (M_D) or ACT copy + TT (M_A).
  T2/fc1 : transpose -> f_all [81, 1024] (ones row 80 -> fc1 bias);
           fc1o = ACT Relu(psf1), fp16.
  fc2    : psf2 [128, 10]; DVE add of f32 (fc2_b - 10) -> t1_all.
  softmax: one batched epilogue per core: Exp(80) + 6D reduce_sum + Ln +
           8 scalar subs; single output DMA.  Act tables load twice total.

dtypes: fp8 only where corrected (conv1 data path); fp16 elsewhere on PE;
fp32 PSUM + f32 fc2 bias add keep log_softmax exact to ~1.6e-2 max rel.
"""

from contextlib import ExitStack

import numpy as np
import ml_dtypes

import concourse.mybir as mybir
import concourse.tile as tile
from concourse import bacc
from concourse.bass_utils import run_bass_kernel_spmd

F32 = mybir.dt.float32
FP16 = mybir.dt.float16
FP8 = mybir.dt.float8e4
AF = mybir.ActivationFunctionType
AX = mybir.AxisListType
DR = mybir.MatmulPerfMode.DoubleRow
E4 = ml_dtypes.float8_e4m3

N_CORES = 8
B_TOTAL = 8192
B_CORE = B_TOTAL // N_CORES  # 1024
S1 = 32.0

# pool mode tables, tuned against TimelineSim: 'A' = ACT-copy path,
# 'D' = DVE direct reduce.  MODE1: 6 window-pairs/block; MODE2: 4 row-pairs.
MODE1 = ["ADADAD", "ADAADA"]  # alternating per block parity
MODE2 = ["DDDD", "DDDD"]


# --------------------------------------------------------------------------
# Host-side weight preparation
# --------------------------------------------------------------------------
def _q8(a):
    return a.astype(E4)


def _prep_weights(mask_w, conv1_w, conv1_b, conv2_w, conv2_b, fc1_w, fc1_b,
                  fc2_w, fc2_b):
    f32 = np.float32
    sig = (1.0 / (1.0 + np.exp(-mask_w.astype(f32)))).astype(f32)  # [28,28]

    # conv1 Toeplitz per window t=(w,h): [128, 480], n = (tr,tc,u,o,m)
    # out row p = 4w + 2u + tr, col q = 12h + 2m + tc; k = r*16 + c
    # value = conv1_w[o,0,ki,kj] * sig[p+ki, q+kj], ki=r-dp, kj=c-ql
    w1m = np.zeros((128, 12 * 960), E4)
    oo = np.arange(10)
    for w in range(6):
        for h in range(2):
            t = w * 2 + h
            wt = np.zeros((128, 480), f32)
            for u in range(2):
                for tr in range(2):
                    dp = 2 * u + tr
                    for m in range(6):
                        for tc in range(2):
                            ql = 2 * m + tc
                            for ki in range(5):
                                r = dp + ki
                                for kj in range(5):
                                    c = ql + kj
                                    n = tr * 240 + tc * 120 + u * 60 + oo * 6 + m
                                    wt[r * 16 + c, n] = (
                                        conv1_w[:, 0, ki, kj]
                                        * sig[4 * w + r, 12 * h + c])
            wq = _q8(wt * S1)
            slot1 = wq.copy()
            # bias value depends only on o: col n -> o = (n % 60) // 6
            nidx = np.arange(480)
            bias_n = conv1_b.astype(f32)[(nidx % 60) // 6] * S1
            slot1[127, :] = _q8(bias_n)
            for c2 in range(2):
                base = t * 960 + c2 * 480
                w1m[:, base:base + 240] = wq[:, c2 * 240:(c2 + 1) * 240]
                w1m[:, base + 240:base + 480] = slot1[:, c2 * 240:
                                                      (c2 + 1) * 240]

    # conv2 single-row Toeplitz: per ki [121, 160], n2 = tc*80 + o*4 + s
    # (q2 = 2s + tc); row (c*12 + j) = conv2_w[o,c,ki,j-q2]/S1; row 120 =
    # b2[o] for ki==0.
    w2m = np.zeros((121, 800), np.float16)
    for ki in range(5):
        blk = np.zeros((121, 160), f32)
        for c in range(10):
            for j in range(12):
                for o in range(20):
                    for s in range(4):
                        for tc in range(2):
                            q2 = 2 * s + tc
                            kj = j - q2
                            if 0 <= kj < 5:
                                blk[c * 12 + j, tc * 80 + o * 4 + s] = \
                                    conv2_w[o, c, ki, kj] / S1
        if ki == 0:
            o_of = (np.arange(160) % 80) // 4
            blk[120, :] = conv2_b.astype(f32)[o_of]
        w2m[:, ki * 160:(ki + 1) * 160] = blk.astype(np.float16)

    # fc1 weights per pooled-row group p': [81, 200]; row 80 = fc1_b (g0)
    fc1w4 = fc1_w.reshape(50, 20, 4, 4)  # [m, o2, p', s2]
    wfc1 = np.zeros((81, 200), np.float16)
    for p in range(4):
        wfc1[0:80, p * 50:(p + 1) * 50] = \
            fc1w4[:, :, p, :].reshape(50, 80).T.astype(np.float16)
    wfc1[80, 0:50] = fc1_b.astype(np.float16)

    # wfcb fp16 [81, 210]: fc2_w.T | wfc1
    wfcb = np.zeros((81, 210), np.float16)
    wfcb[0:50, 0:10] = fc2_w.T.astype(np.float16)
    wfcb[:, 10:210] = wfc1

    # cst f32 [128, 20]: doubled (fc2_b - 10)
    cst = np.tile(np.concatenate([fc2_b.astype(f32) - 10.0] * 2)
                  .reshape(1, 20), (128, 1)).astype(f32)

    idb = np.eye(128).astype(np.float16)
    return dict(w1m=w1m, w2m=w2m, wfcb=wfcb, cst=cst, idb=idb)


# --------------------------------------------------------------------------
# Device program
# --------------------------------------------------------------------------
def _build(b_core):
    assert b_core % 256 == 0
    n_pair = b_core // 256

    nc = bacc.Bacc("TRN2", target_bir_lowering=False, debug=False,
                   num_devices=N_CORES)

    xw_d = nc.dram_tensor("xw", [12, 128, 2 * b_core], FP8,
                          kind="ExternalInput").ap()
    w1m_d = nc.dram_tensor("w1m", [128, 11520], FP8,
                           kind="ExternalInput").ap()
    w2m_d = nc.dram_tensor("w2m", [121, 800], FP16, kind="ExternalInput").ap()
    wfcb_d = nc.dram_tensor("wfcb", [81, 210], FP16,
                            kind="ExternalInput").ap()
    cst_d = nc.dram_tensor("cst", [128, 20], F32, kind="ExternalInput").ap()
    idb_d = nc.dram_tensor("idb", [128, 128], FP16, kind="ExternalInput").ap()
    y = nc.dram_tensor("y", [b_core, 10], F32, kind="ExternalOutput").ap()

    MAX, ADD = mybir.AluOpType.max, mybir.AluOpType.add

    with tile.TileContext(nc) as tc, ExitStack() as ctx:
        consts = ctx.enter_context(tc.tile_pool(name="consts", bufs=1))
        identb = consts.tile([128, 128], FP16)
        nc.sync.dma_start(identb[:], idb_d)
        w1m_sb = consts.tile([128, 11520], FP8)
        w2m_sb = consts.tile([121, 800], FP16)
        wfcb_sb = consts.tile([81, 210], FP16)
        cst_sb = consts.tile([128, 20], F32)

        wfc2_sb = wfcb_sb[0:50, 0:10]
        wfc1_sb = wfcb_sb[:, 10:210]
        t1_all = consts.tile([128, 10 * 8], F32)

        xw_pool = ctx.enter_context(tc.tile_pool(name="xw", bufs=3))
        ps1_pool = ctx.enter_context(tc.tile_pool(name="ps1", bufs=2,
                                                  space="PSUM"))
        tmp_pool = ctx.enter_context(tc.tile_pool(name="tmpb", bufs=4))
        prp_pool = ctx.enter_context(tc.tile_pool(name="prp", bufs=8))
        tpw_pool = ctx.enter_context(tc.tile_pool(name="tpw", bufs=2,
                                                  space="PSUM"))
        x2_pool = ctx.enter_context(tc.tile_pool(name="x2", bufs=6))
        ps2_pool = ctx.enter_context(tc.tile_pool(name="ps2", bufs=2,
                                                  space="PSUM"))
        p2_pool = ctx.enter_context(tc.tile_pool(name="p2", bufs=5))
        f_pool = ctx.enter_context(tc.tile_pool(name="fp", bufs=2))
        fc1o_pool = ctx.enter_context(tc.tile_pool(name="fc1o", bufs=2))
        sm_pool = ctx.enter_context(tc.tile_pool(name="sm", bufs=1))

        n_blk = 2 * n_pair
        xw_tiles = {}
        state = {}      # blk -> (prp_t list, xwcat ref)
        f_cur = [None]  # f_all tile for the current pair

        def fetch_pair(p, split=False):
            if p >= n_pair or p in xw_tiles:
                return
            xwcat = xw_pool.tile([128, 12 * 512], FP8, name="xwcat", tag="xw")
            deng = nc.sync if p % 2 == 0 else nc.scalar
            src = xw_d[:, :, p * 512:(p + 1) * 512]
            dst = xwcat.rearrange("p (t c) -> p t c", t=12)
            if split:
                deng.dma_start(dst[:, 0:4], src[0:4].rearrange(
                    "t p c -> p t c"))
                deng.dma_start(dst[:, 4:12], src[4:12].rearrange(
                    "t p c -> p t c"))
            else:
                deng.dma_start(dst, src.rearrange("t p c -> p t c"))
            xw_tiles[p] = xwcat

        def stage_a(blk):
            """conv1 (fp8 DR) + pool1 per window pair."""
            pair, half = blk // 2, blk % 2
            xwcat = xw_tiles[pair]
            mode1 = MODE1[blk % 2]
            prp_t = []
            for wp in range(6):
                pst = ps1_pool.tile([128, 1024], F32, name="pst", tag="ps1")
                for wi in range(2):
                    t = wp * 2 + wi
                    lhsT = xwcat[:, t * 512 + half * 256:
                                 t * 512 + half * 256 + 256] \
                        .rearrange("p (two m) -> p two m", two=2)
                    for c2 in range(2):
                        nc.tensor.matmul(
                            pst[:, wi * 512 + c2 * 240:
                                wi * 512 + c2 * 240 + 240],
                            lhsT,
                            w1m_sb[:, t * 960 + c2 * 480:
                                   t * 960 + (c2 + 1) * 480]
                            .rearrange("p (two f) -> p two f", two=2),
                            start=True, stop=True, perf_mode=DR)
                # pooling: psum n-order per window = (tr, tc, u, o, m),
                # windows at 512-elem offsets (480 used, 32 pad)
                prp = prp_pool.tile([128, 240], FP16, name="prp", tag="prp")
                prp_t.append(prp)
                # prp col order (u, o, h, m): u:120, o:12, h:6, m:1
                prp_v = prp.rearrange("p (u o h m) -> p h (u o) m",
                                      u=2, o=10, h=2)
                pwin = pst.rearrange("p (h z) -> p h z", h=2)[:, :, 0:480]
                if mode1[wp] == "D":
                    src6 = pwin.rearrange(
                        "p h (tr tc uo m) -> p h uo m tr tc",
                        tr=2, tc=2, uo=20)
                    nc.vector.reduce_max(prp_v, src6, axis=AX.XY)
                else:
                    tmp = tmp_pool.tile([128, 960], FP16, name="tmp",
                                        tag="tmp")
                    nc.scalar.activation(
                        tmp.rearrange("p (h f) -> p h f", h=2), pwin,
                        AF.Relu)
                    tv = tmp.rearrange("p (h tr f) -> p h tr f", h=2, tr=2)
                    rm = tmp_pool.tile([128, 480], FP16, name="rm", tag="rm")
                    rmv = rm.rearrange("p (h f) -> p h f", h=2)
                    nc.vector.tensor_max(rmv, tv[:, :, 0], tv[:, :, 1])
                    rv5 = rm.rearrange("p (h tc uo m) -> p h tc uo m",
                                       h=2, tc=2, uo=20)
                    nc.vector.tensor_max(prp_v, rv5[:, :, 0], rv5[:, :, 1])
            state[blk] = prp_t
            if half == 1:
                fetch_pair(pair + 1)
            if blk == 0:
                nc.sync.dma_start(w2m_sb[:], w2m_d)
                nc.scalar.dma_start(wfcb_sb[:], wfcb_d)
                nc.scalar.dma_start(cst_sb[:], cst_d)

        def stage_b(blk):
            """T1 + evict -> conv2 + pool2 -> T2 -> f_all; fc on odd blk."""
            pair, half = blk // 2, blk % 2
            prp_t = state.pop(blk)
            mode2 = MODE2[blk % 2]
            if half == 0:
                f_all = f_pool.tile([81, 1024], FP16, name="f_all",
                                    tag="f_all")
                nc.gpsimd.memset(f_all[80:81, :], 1.0)
                f_cur[0] = f_all
            f_all = f_cur[0]

            # ---- T1 transposes + relu evict -> x2cat ----
            x2cat = []
            for ww in range(3):
                tpw = tpw_pool.tile([120, 512], FP16, name="tpw", tag="tpw")
                for q in range(4):
                    r = ww * 4 + q          # pooled row 0..11
                    prp = prp_t[r // 2]
                    u = r % 2
                    nc.tensor.transpose(
                        tpw[:, q * 128:(q + 1) * 128],
                        prp[:, u * 120:u * 120 + 120], identb[:])
                x2c = x2_pool.tile([121, 512], FP16, name="x2c", tag="x2c")
                nc.gpsimd.memset(x2c[120:121, :], 1.0)
                nc.scalar.activation(x2c[0:120, :], tpw[:], AF.Relu)
                x2cat.append(x2c)

            # ---- conv2 (fp16 single-row) + pool2 + T2 ----
            tp2w = tpw_pool.tile([80, 512], FP16, name="tp2w", tag="tpw")
            for g in range(4):
                ps2 = ps2_pool.tile([128, 320], F32, name="ps2", tag="ps2")
                for sub in range(2):
                    p2r = g * 2 + sub
                    for ki in range(5):
                        i = p2r + ki
                        nc.tensor.matmul(
                            ps2[:, sub * 160:sub * 160 + 160],
                            x2cat[i // 4][0:121,
                                          (i % 4) * 128:(i % 4 + 1) * 128],
                            w2m_sb[:, ki * 160:(ki + 1) * 160],
                            start=(ki == 0), stop=(ki == 4))
                # pool2: region layout n2 = (tc, o, s); rows = pl
                p2t = p2_pool.tile([128, 80], FP16, name="p2t", tag="p2t")
                if mode2[g] == "D":
                    src = ps2.rearrange("p (pl tc os) -> p os pl tc",
                                        pl=2, tc=2)
                    nc.vector.reduce_max(p2t[:], src, axis=AX.XY)
                else:
                    tmp2 = tmp_pool.tile([128, 320], FP16, name="tmp2",
                                         tag="tmp2")
                    nc.scalar.activation(tmp2[:], ps2[:], AF.Relu)
                    t2v = tmp2.rearrange("p (pl f) -> p pl f", pl=2)
                    rm2 = tmp_pool.tile([128, 160], FP16, name="rm2",
                                        tag="rm2")
                    nc.vector.tensor_max(rm2[:], t2v[:, 0], t2v[:, 1])
                    r2v = rm2.rearrange("p (tc f) -> p tc f", tc=2)
                    nc.vector.tensor_max(p2t[:], r2v[:, 0], r2v[:, 1])
                nc.tensor.transpose(tp2w[:, g * 128:(g + 1) * 128],
                                    p2t[:], identb[:])
            nc.scalar.activation(f_all[0:80, half * 512:half * 512 + 512],
                                 tp2w[:], AF.Relu)
            if half == 0:
                return

            # ---- fc1 over the 256-sample pair ----
            psf1 = ps2_pool.tile([50, 256], F32, name="psf1", tag="ps2")
            for g in range(4):
                rows = 81 if g == 0 else 80
                fvg = f_all[0:rows, :].rearrange("p (h g n) -> p g h n",
                                                 h=2, g=4)[:, g]
                nc.tensor.matmul(psf1[:], wfc1_sb[0:rows, g * 50:g * 50 + 50],
                                 fvg, start=(g == 0), stop=(g == 3))
            fc1o = fc1o_pool.tile([50, 256], FP16, name="fc1o", tag="fc1o")
            nc.scalar.activation(fc1o[:], psf1[:], AF.Relu)
            # ---- fc2 ----
            psf2 = ps2_pool.tile([128, 20], F32, name="psf2", tag="ps2")
            for h2 in range(2):
                nc.tensor.matmul(psf2[:, h2 * 10:h2 * 10 + 10],
                                 fc1o[:, h2 * 128:h2 * 128 + 128],
                                 wfc2_sb[:], start=True, stop=True)
            nc.vector.tensor_add(t1_all[:, pair * 20:pair * 20 + 20],
                                 psf2[:], cst_sb[:])
            # per-pair Exp + sums keep the epilogue tail short (Exp is in
            # the same act-func set as Relu, so no extra table loads)
            nc.scalar.activation(e_all[:, pair * 20:pair * 20 + 20],
                                 t1_all[:, pair * 20:pair * 20 + 20], AF.Exp)
            nc.vector.reduce_sum(
                se[:, pair * 2:pair * 2 + 2],
                e_all[:, pair * 20:pair * 20 + 20]
                .rearrange("p (b t) -> p b t", t=10), axis=AX.X)

        # software-pipelined emission: stage_b(blk-1) before stage_a(blk).
        # startup DMA order minimizes time-to-first-matmul given the
        # serialized HWDGE (~630ns per DMA issue): first 4 windows of x,
        # then their weights, then the rest, each as one large DMA.
        fetch_pair(0, split=True)
        nc.scalar.dma_start(w1m_sb[:, 0:4 * 960], w1m_d[:, 0:4 * 960])
        nc.sync.dma_start(w1m_sb[:, 4 * 960:], w1m_d[:, 4 * 960:])
        e_all = sm_pool.tile([128, 80], F32, name="e_all", tag="e_all")
        se = sm_pool.tile([128, 8], F32, name="se", tag="se")
        for it in range(n_blk + 1):
            if it >= 1:
                stage_b(it - 1)
            if it < n_blk:
                stage_a(it)

        # ---- log_softmax tail: one Ln + subs ----
        ls = sm_pool.tile([128, 8], F32, name="ls", tag="ls")
        nc.scalar.activation(ls[:], se[:], AF.Ln)
        yo = sm_pool.tile([128, 80], F32, name="yo", tag="yo")
        for b in range(8):
            nc.vector.tensor_scalar_sub(yo[:, b * 10:b * 10 + 10],
                                        t1_all[:, b * 10:b * 10 + 10],
                                        ls[:, b:b + 1])
        nc.scalar.dma_start(
            y.rearrange("(blk p) c -> p blk c", p=128),
            yo.rearrange("p (blk c) -> p blk c", c=10))

    nc.compile()
    return nc


_PROGRAM_CACHE = {}


def _get_program(b_core):
    if b_core not in _PROGRAM_CACHE:
        _PROGRAM_CACHE[b_core] = _build(b_core)
    return _PROGRAM_CACHE[b_core]


def make_in_maps(x, weights, b_core=B_CORE, n_cores=N_CORES):
    """Shard x over cores; replicate the (rearranged) parameters."""
    f32 = np.float32
    xr = np.asarray(x, dtype=f32).reshape(-1, 28, 28)
    in_maps = []
    for cidx in range(n_cores):
        xc = xr[cidx * b_core:(cidx + 1) * b_core]  # [b_core, 28, 28]
        xwin = np.zeros((12, 128, 2 * b_core), E4)
        for w in range(6):
            for h in range(2):
                t = w * 2 + h
                win = xc[:, 4 * w:4 * w + 8, 12 * h:12 * h + 16] \
                    .reshape(b_core, 128).astype(f32)
                hi = win.astype(E4)
                lo = (win - hi.astype(f32)).astype(E4)
                lo_f = lo.astype(f32)
                lo_f[:, 127] = 1.0
                lo = lo_f.astype(E4)
                hiT = hi.astype(f32).T.astype(E4)   # [128, b_core]
                loT = lo.astype(f32).T.astype(E4)
                for blk in range(b_core // 128):
                    p, hb = blk // 2, blk % 2
                    base = p * 512 + hb * 256
                    xwin[t, :, base:base + 128] = \
                        hiT[:, blk * 128:(blk + 1) * 128]
                    xwin[t, :, base + 128:base + 256] = \
                        loT[:, blk * 128:(blk + 1) * 128]
        m = {"xw": xwin}
        m.update(weights)
        in_maps.append(m)
    return in_maps


def kernel(**inputs):
    x = np.asarray(inputs["x"], dtype=np.float32)
    weights = _prep_weights(
        np.asarray(inputs["mask_w"], np.float32),
        np.asarray(inputs["conv1_w"], np.float32),
        np.asarray(inputs["conv1_b"], np.float32),
        np.asarray(inputs["conv2_w"], np.float32),
        np.asarray(inputs["conv2_b"], np.float32),
        np.asarray(inputs["fc1_w"], np.float32),
        np.asarray(inputs["fc1_b"], np.float32),
        np.asarray(inputs["fc2_w"], np.float32),
        np.asarray(inputs["fc2_b"], np.float32),
    )
    nc = _get_program(B_CORE)
    in_maps = make_in_maps(x, weights)
    res = run_bass_kernel_spmd(nc, in_maps, list(range(N_CORES)))
    out = np.concatenate([res.results[c]["y"] for c in range(N_CORES)],
                         axis=0)
    return np.ascontiguousarray(out.astype(np.float32))


if __name__ == "__main__":
    rng = np.random.default_rng(0)
    ins = {
        "x": rng.standard_normal((B_TOTAL, 1, 28, 28), dtype=np.float32),
        "mask_w": rng.standard_normal((28, 28), dtype=np.float32) * 0.1,
        "conv1_w": rng.standard_normal((10, 1, 5, 5), dtype=np.float32) * 0.2,
        "conv1_b": rng.standard_normal((10,), dtype=np.float32) * 0.1,
        "conv2_w": rng.standard_normal((20, 10, 5, 5),
                                       dtype=np.float32) * 0.06,
        "conv2_b": rng.standard_normal((20,), dtype=np.float32) * 0.1,
        "fc1_w": rng.standard_normal((50, 320), dtype=np.float32) * 0.05,
        "fc1_b": rng.standard_normal((50,), dtype=np.float32) * 0.1,
        "fc2_w": rng.standard_normal((10, 50), dtype=np.float32) * 0.14,
        "fc2_b": rng.standard_normal((10,), dtype=np.float32) * 0.1,
    }
    out = kernel(**ins)
    print(out.shape, out.dtype, out[:2])


# revision 16
# speedup vs baseline: 1.0541x; 1.0392x over previous
"""Trainium2 Bass kernel for nn_CNN_Casual (LeNet-ish CNN, B=8192). v2.

Pure data parallel over 8 NeuronCores: 1024 samples/core, 8 blocks of 128.

Key structure (vs baseline):
  conv1  : fp8-e4m3 DoubleRow matmuls, data-corrected: stationary holds
           (x_hi, x_lo) planes (x = x_hi + x_lo, both e4m3; exact to ~2^-8),
           moving holds the masked Toeplitz weights (e4m3, scale S1)
           duplicated into both pair slots.  2 windows share one 2-bank
           psum tile; 2 chunk matmuls per window at 0.5 cycles/column.
           conv1 bias rides the sacrificial lo-slot row 127 (corner pixel
           lo-bits dropped; ~0.3% of one tap).
  pool1  : per window-pair, either
             M_D: one fused 6D reduce_max (4:1) from psum (DVE only), or
             M_A: ACT relu-copy psum->fp16 + two packed-fp16 2x tensor_max
           chosen per-wpair to balance DVE vs ACT.
  T1     : PE transposes into [120,512] psum; ACT Relu evict -> x2cat fp16
           [121, 512] (row 120 = ones for the conv2 bias row).
  conv2  : fp16, single-output-row accumulation: 8 rows x 5 matmuls
           [121 x 160], bias via the ones row (b2 in W row 120 of ki=0).
  pool2  : per row-pair: fused 4D reduce_max (M_D) or ACT copy + TT (M_A).
  T2/fc1 : transpose -> f_all [81, 1024] (ones row 80 -> fc1 bias);
           fc1o = ACT Relu(psf1), fp16.
  fc2    : psf2 [128, 10]; DVE add of f32 (fc2_b - 10) -> t1_all.
  softmax: one batched epilogue per core: Exp(80) + 6D reduce_sum + Ln +
           8 scalar subs; single output DMA.  Act tables load twice total.

dtypes: fp8 only where corrected (conv1 data path); fp16 elsewhere on PE;
fp32 PSUM + f32 fc2 bias add keep log_softmax exact to ~1.6e-2 max rel.
"""

from contextlib import ExitStack

import numpy as np
import ml_dtypes

import concourse.mybir as mybir
import concourse.tile as tile
from concourse import bacc
from concourse.bass_utils import run_bass_kernel_spmd

F32 = mybir.dt.float32
FP16 = mybir.dt.float16
FP8 = mybir.dt.float8e4
AF = mybir.ActivationFunctionType
AX = mybir.AxisListType
DR = mybir.MatmulPerfMode.DoubleRow
E4 = ml_dtypes.float8_e4m3

N_CORES = 8
B_TOTAL = 8192
B_CORE = B_TOTAL // N_CORES  # 1024
S1 = 32.0

# pool mode tables, tuned against TimelineSim: 'A' = ACT-copy path,
# 'D' = DVE direct reduce.  MODE1: 6 window-pairs/block; MODE2: 4 row-pairs.
MODE1 = ["ADADAD", "ADAADA"]  # alternating per block parity
MODE2 = ["DDDD", "DDDD"]


# --------------------------------------------------------------------------
# Host-side weight preparation
# --------------------------------------------------------------------------
def _q8(a):
    return a.astype(E4)


def _prep_weights(mask_w, conv1_w, conv1_b, conv2_w, conv2_b, fc1_w, fc1_b,
                  fc2_w, fc2_b):
    f32 = np.float32
    sig = (1.0 / (1.0 + np.exp(-mask_w.astype(f32)))).astype(f32)  # [28,28]

    # conv1 Toeplitz per window t=(w,h): [128, 480], n = (tr,tc,u,o,m)
    # out row p = 4w + 2u + tr, col q = 12h + 2m + tc; k = r*16 + c
    # value = conv1_w[o,0,ki,kj] * sig[p+ki, q+kj], ki=r-dp, kj=c-ql
    w1m = np.zeros((128, 12 * 960), E4)
    oo = np.arange(10)
    for w in range(6):
        for h in range(2):
            t = w * 2 + h
            wt = np.zeros((128, 480), f32)
            for u in range(2):
                for tr in range(2):
                    dp = 2 * u + tr
                    for m in range(6):
                        for tc in range(2):
                            ql = 2 * m + tc
                            for ki in range(5):
                                r = dp + ki
                                for kj in range(5):
                                    c = ql + kj
                                    n = tr * 240 + tc * 120 + u * 60 + oo * 6 + m
                                    wt[r * 16 + c, n] = (
                                        conv1_w[:, 0, ki, kj]
                                        * sig[4 * w + r, 12 * h + c])
            wq = _q8(wt * S1)
            slot1 = wq.copy()
            # bias value depends only on o: col n -> o = (n % 60) // 6
            nidx = np.arange(480)
            bias_n = conv1_b.astype(f32)[(nidx % 60) // 6] * S1
            slot1[127, :] = _q8(bias_n)
            for c2 in range(2):
                base = t * 960 + c2 * 480
                w1m[:, base:base + 240] = wq[:, c2 * 240:(c2 + 1) * 240]
                w1m[:, base + 240:base + 480] = slot1[:, c2 * 240:
                                                      (c2 + 1) * 240]

    # conv2 single-row Toeplitz: per ki [121, 160], n2 = tc*80 + o*4 + s
    # (q2 = 2s + tc); row (c*12 + j) = conv2_w[o,c,ki,j-q2]/S1; row 120 =
    # b2[o] for ki==0.
    w2m = np.zeros((121, 800), np.float16)
    for ki in range(5):
        blk = np.zeros((121, 160), f32)
        for c in range(10):
            for j in range(12):
                for o in range(20):
                    for s in range(4):
                        for tc in range(2):
                            q2 = 2 * s + tc
                            kj = j - q2
                            if 0 <= kj < 5:
                                blk[c * 12 + j, tc * 80 + o * 4 + s] = \
                                    conv2_w[o, c, ki, kj] / S1
        if ki == 0:
            o_of = (np.arange(160) % 80) // 4
            blk[120, :] = conv2_b.astype(f32)[o_of]
        w2m[:, ki * 160:(ki + 1) * 160] = blk.astype(np.float16)

    # fc1 weights per pooled-row group p': [81, 200]; row 80 = fc1_b (g0)
    fc1w4 = fc1_w.reshape(50, 20, 4, 4)  # [m, o2, p', s2]
    wfc1 = np.zeros((81, 200), np.float16)
    for p in range(4):
        wfc1[0:80, p * 50:(p + 1) * 50] = \
            fc1w4[:, :, p, :].reshape(50, 80).T.astype(np.float16)
    wfc1[80, 0:50] = fc1_b.astype(np.float16)

    # wfcb fp16 [81, 210]: fc2_w.T | wfc1
    wfcb = np.zeros((81, 210), np.float16)
    wfcb[0:50, 0:10] = fc2_w.T.astype(np.float16)
    wfcb[:, 10:210] = wfc1

    # cst f32 [128, 20]: doubled (fc2_b - 10)
    cst = np.tile(np.concatenate([fc2_b.astype(f32) - 10.0] * 2)
                  .reshape(1, 20), (128, 1)).astype(f32)

    idb = np.eye(128).astype(np.float16)
    return dict(w1m=w1m, w2m=w2m, wfcb=wfcb, cst=cst, idb=idb)


# --------------------------------------------------------------------------
# Device program
# --------------------------------------------------------------------------
def _build(b_core):
    assert b_core % 256 == 0
    n_pair = b_core // 256

    nc = bacc.Bacc("TRN2", target_bir_lowering=False, debug=False,
                   num_devices=N_CORES)

    xw_d = nc.dram_tensor("xw", [12, 128, 2 * b_core], FP8,
                          kind="ExternalInput").ap()
    w1m_d = nc.dram_tensor("w1m", [128, 11520], FP8,
                           kind="ExternalInput").ap()
    w2m_d = nc.dram_tensor("w2m", [121, 800], FP16, kind="ExternalInput").ap()
    wfcb_d = nc.dram_tensor("wfcb", [81, 210], FP16,
                            kind="ExternalInput").ap()
    cst_d = nc.dram_tensor("cst", [128, 20], F32, kind="ExternalInput").ap()
    idb_d = nc.dram_tensor("idb", [128, 128], FP16, kind="ExternalInput").ap()
    y = nc.dram_tensor("y", [b_core, 10], F32, kind="ExternalOutput").ap()

    MAX, ADD = mybir.AluOpType.max, mybir.AluOpType.add

    with tile.TileContext(nc) as tc, ExitStack() as ctx:
        consts = ctx.enter_context(tc.tile_pool(name="consts", bufs=1))
        identb = consts.tile([128, 128], FP16)
        nc.sync.dma_start(identb[:], idb_d)
        w1m_sb = consts.tile([128, 11520], FP8)
        w2m_sb = consts.tile([121, 800], FP16)
        wfcb_sb = consts.tile([81, 210], FP16)
        cst_sb = consts.tile([128, 20], F32)

        wfc2_sb = wfcb_sb[0:50, 0:10]
        wfc1_sb = wfcb_sb[:, 10:210]
        t1_all = consts.tile([128, 10 * 8], F32)

        xw_pool = ctx.enter_context(tc.tile_pool(name="xw", bufs=3))
        ps1_pool = ctx.enter_context(tc.tile_pool(name="ps1", bufs=2,
                                                  space="PSUM"))
        tmp_pool = ctx.enter_context(tc.tile_pool(name="tmpb", bufs=4))
        prp_pool = ctx.enter_context(tc.tile_pool(name="prp", bufs=8))
        tpw_pool = ctx.enter_context(tc.tile_pool(name="tpw", bufs=2,
                                                  space="PSUM"))
        x2_pool = ctx.enter_context(tc.tile_pool(name="x2", bufs=6))
        ps2_pool = ctx.enter_context(tc.tile_pool(name="ps2", bufs=2,
                                                  space="PSUM"))
        p2_pool = ctx.enter_context(tc.tile_pool(name="p2", bufs=5))
        f_pool = ctx.enter_context(tc.tile_pool(name="fp", bufs=2))
        fc1o_pool = ctx.enter_context(tc.tile_pool(name="fc1o", bufs=2))
        sm_pool = ctx.enter_context(tc.tile_pool(name="sm", bufs=1))

        n_blk = 2 * n_pair
        xw_tiles = {}
        state = {}      # blk -> (prp_t list, xwcat ref)
        f_cur = [None]  # f_all tile for the current pair

        def fetch_pair(p, split=False):
            if p >= n_pair or p in xw_tiles:
                return
            xwcat = xw_pool.tile([128, 12 * 512], FP8, name="xwcat", tag="xw")
            deng = nc.sync if p % 2 == 0 else nc.scalar
            src = xw_d[:, :, p * 512:(p + 1) * 512]
            dst = xwcat.rearrange("p (t c) -> p t c", t=12)
            if split:
                deng.dma_start(dst[:, 0:4], src[0:4].rearrange(
                    "t p c -> p t c"))
                deng.dma_start(dst[:, 4:12], src[4:12].rearrange(
                    "t p c -> p t c"))
            else:
                deng.dma_start(dst, src.rearrange("t p c -> p t c"))
            xw_tiles[p] = xwcat

        def conv1_wpair(blk, wp, prp_t):
            """conv1 (fp8 DR) + pool1 for one window pair of block blk."""
            pair, half = blk // 2, blk % 2
            xwcat = xw_tiles[pair]
            mode1 = MODE1[blk % 2]
            pst = ps1_pool.tile([128, 1024], F32, name="pst", tag="ps1")
            for wi in range(2):
                t = wp * 2 + wi
                lhsT = xwcat[:, t * 512 + half * 256:
                             t * 512 + half * 256 + 256] \
                    .rearrange("p (two m) -> p two m", two=2)
                for c2 in range(2):
                    nc.tensor.matmul(
                        pst[:, wi * 512 + c2 * 240:
                            wi * 512 + c2 * 240 + 240],
                        lhsT,
                        w1m_sb[:, t * 960 + c2 * 480:
                               t * 960 + (c2 + 1) * 480]
                        .rearrange("p (two f) -> p two f", two=2),
                        start=True, stop=True, perf_mode=DR)
            # pooling: psum n-order per window = (tr, tc, u, o, m),
            # windows at 512-elem offsets (480 used, 32 pad)
            prp = prp_pool.tile([128, 240], FP16, name="prp", tag="prp")
            prp_t.append(prp)
            # prp col order (u, o, h, m): u:120, o:12, h:6, m:1
            prp_v = prp.rearrange("p (u o h m) -> p h (u o) m",
                                  u=2, o=10, h=2)
            pwin = pst.rearrange("p (h z) -> p h z", h=2)[:, :, 0:480]
            if mode1[wp] == "D":
                src6 = pwin.rearrange(
                    "p h (tr tc uo m) -> p h uo m tr tc",
                    tr=2, tc=2, uo=20)
                nc.vector.reduce_max(prp_v, src6, axis=AX.XY)
            else:
                tmp = tmp_pool.tile([128, 960], FP16, name="tmp", tag="tmp")
                nc.scalar.activation(
                    tmp.rearrange("p (h f) -> p h f", h=2), pwin, AF.Relu)
                tv = tmp.rearrange("p (h tr f) -> p h tr f", h=2, tr=2)
                rm = tmp_pool.tile([128, 480], FP16, name="rm", tag="rm")
                rmv = rm.rearrange("p (h f) -> p h f", h=2)
                nc.vector.tensor_max(rmv, tv[:, :, 0], tv[:, :, 1])
                rv5 = rm.rearrange("p (h tc uo m) -> p h tc uo m",
                                   h=2, tc=2, uo=20)
                nc.vector.tensor_max(prp_v, rv5[:, :, 0], rv5[:, :, 1])

        def t1_stage(blk, prp_t):
            """T1 transposes + relu evict -> x2cat list."""
            pair, half = blk // 2, blk % 2
            if half == 0:
                f_all = f_pool.tile([81, 1024], FP16, name="f_all",
                                    tag="f_all")
                nc.gpsimd.memset(f_all[80:81, :], 1.0)
                f_cur[0] = f_all
            x2cat = []
            for ww in range(3):
                tpw = tpw_pool.tile([120, 512], FP16, name="tpw", tag="tpw")
                for q in range(4):
                    r = ww * 4 + q          # pooled row 0..11
                    prp = prp_t[r // 2]
                    u = r % 2
                    nc.tensor.transpose(
                        tpw[:, q * 128:(q + 1) * 128],
                        prp[:, u * 120:u * 120 + 120], identb[:])
                x2c = x2_pool.tile([121, 512], FP16, name="x2c", tag="x2c")
                nc.gpsimd.memset(x2c[120:121, :], 1.0)
                nc.scalar.activation(x2c[0:120, :], tpw[:], AF.Relu)
                x2cat.append(x2c)
            return x2cat

        def conv2_grp(blk, g, x2cat, tp2w):
            """conv2 rows 2g, 2g+1 + pool2 + T2 transpose."""
            mode2 = MODE2[blk % 2]
            ps2 = ps2_pool.tile([128, 320], F32, name="ps2", tag="ps2")
            for sub in range(2):
                p2r = g * 2 + sub
                for ki in range(5):
                    i = p2r + ki
                    nc.tensor.matmul(
                        ps2[:, sub * 160:sub * 160 + 160],
                        x2cat[i // 4][0:121,
                                      (i % 4) * 128:(i % 4 + 1) * 128],
                        w2m_sb[:, ki * 160:(ki + 1) * 160],
                        start=(ki == 0), stop=(ki == 4))
            # pool2: region layout n2 = (tc, o, s); rows = pl
            p2t = p2_pool.tile([128, 80], FP16, name="p2t", tag="p2t")
            if mode2[g] == "D":
                src = ps2.rearrange("p (pl tc os) -> p os pl tc",
                                    pl=2, tc=2)
                nc.vector.reduce_max(p2t[:], src, axis=AX.XY)
            else:
                tmp2 = tmp_pool.tile([128, 320], FP16, name="tmp2",
                                     tag="tmp2")
                nc.scalar.activation(tmp2[:], ps2[:], AF.Relu)
                t2v = tmp2.rearrange("p (pl f) -> p pl f", pl=2)
                rm2 = tmp_pool.tile([128, 160], FP16, name="rm2", tag="rm2")
                nc.vector.tensor_max(rm2[:], t2v[:, 0], t2v[:, 1])
                r2v = rm2.rearrange("p (tc f) -> p tc f", tc=2)
                nc.vector.tensor_max(p2t[:], r2v[:, 0], r2v[:, 1])
            nc.tensor.transpose(tp2w[:, g * 128:(g + 1) * 128],
                                p2t[:], identb[:])

        def fc_stage(pair):
            f_all = f_cur[0]
            psf1 = ps2_pool.tile([50, 256], F32, name="psf1", tag="ps2")
            for g in range(4):
                rows = 81 if g == 0 else 80
                fvg = f_all[0:rows, :].rearrange("p (h g n) -> p g h n",
                                                 h=2, g=4)[:, g]
                nc.tensor.matmul(psf1[:], wfc1_sb[0:rows, g * 50:g * 50 + 50],
                                 fvg, start=(g == 0), stop=(g == 3))
            fc1o = fc1o_pool.tile([50, 256], FP16, name="fc1o", tag="fc1o")
            nc.scalar.activation(fc1o[:], psf1[:], AF.Relu)
            psf2 = ps2_pool.tile([128, 20], F32, name="psf2", tag="ps2")
            for h2 in range(2):
                nc.tensor.matmul(psf2[:, h2 * 10:h2 * 10 + 10],
                                 fc1o[:, h2 * 128:h2 * 128 + 128],
                                 wfc2_sb[:], start=True, stop=True)
            nc.vector.tensor_add(t1_all[:, pair * 20:pair * 20 + 20],
                                 psf2[:], cst_sb[:])
            # per-pair Exp + sums keep the epilogue tail short (Exp is in
            # the same act-func set as Relu, so no extra table loads)
            nc.scalar.activation(e_all[:, pair * 20:pair * 20 + 20],
                                 t1_all[:, pair * 20:pair * 20 + 20], AF.Exp)
            nc.vector.reduce_sum(
                se[:, pair * 2:pair * 2 + 2],
                e_all[:, pair * 20:pair * 20 + 20]
                .rearrange("p (b t) -> p b t", t=10), axis=AX.X)

        def emit_iter(b):
            """Interleaved emission: T1(b-1), then conv1(b) window pairs
            woven with conv2(b-1) groups so block b's psum appears early."""
            prev = b - 1
            x2cat = tp2w = None
            if prev >= 0:
                x2cat = t1_stage(prev, state.pop(prev))
                tp2w = tpw_pool.tile([80, 512], FP16, name="tp2w", tag="tpw")
            prp_t = []
            if b < n_blk:
                state[b] = prp_t
            for wp in range(6):
                if b < n_blk:
                    conv1_wpair(b, wp, prp_t)
                if prev >= 0 and b < n_blk and wp in (1, 2, 3, 4):
                    conv2_grp(prev, wp - 1, x2cat, tp2w)
            if prev >= 0:
                if b >= n_blk:
                    for g in range(4):
                        conv2_grp(prev, g, x2cat, tp2w)
                half = prev % 2
                nc.scalar.activation(
                    f_cur[0][0:80, half * 512:half * 512 + 512],
                    tp2w[:], AF.Relu)
                if half == 1:
                    fc_stage(prev // 2)
            if b < n_blk:
                pair, half = b // 2, b % 2
                if half == 1:
                    fetch_pair(pair + 1)
                if b == 0:
                    nc.sync.dma_start(w2m_sb[:], w2m_d)
                    nc.scalar.dma_start(wfcb_sb[:], wfcb_d)
                    nc.scalar.dma_start(cst_sb[:], cst_d)

        # software-pipelined emission: stage_b(blk-1) before stage_a(blk).
        # startup DMA order minimizes time-to-first-matmul given the
        # serialized HWDGE (~630ns per DMA issue): first 4 windows of x,
        # then their weights, then the rest, each as one large DMA.
        fetch_pair(0, split=True)
        nc.scalar.dma_start(w1m_sb[:, 0:4 * 960], w1m_d[:, 0:4 * 960])
        nc.sync.dma_start(w1m_sb[:, 4 * 960:], w1m_d[:, 4 * 960:])
        e_all = sm_pool.tile([128, 80], F32, name="e_all", tag="e_all")
        se = sm_pool.tile([128, 8], F32, name="se", tag="se")
        for it in range(n_blk + 1):
            emit_iter(it)

        # ---- log_softmax tail: one Ln + subs ----
        ls = sm_pool.tile([128, 8], F32, name="ls", tag="ls")
        nc.scalar.activation(ls[:], se[:], AF.Ln)
        yo = sm_pool.tile([128, 80], F32, name="yo", tag="yo")
        for b in range(8):
            nc.vector.tensor_scalar_sub(yo[:, b * 10:b * 10 + 10],
                                        t1_all[:, b * 10:b * 10 + 10],
                                        ls[:, b:b + 1])
        nc.scalar.dma_start(
            y.rearrange("(blk p) c -> p blk c", p=128),
            yo.rearrange("p (blk c) -> p blk c", c=10))

    nc.compile()
    return nc


_PROGRAM_CACHE = {}


def _get_program(b_core):
    if b_core not in _PROGRAM_CACHE:
        _PROGRAM_CACHE[b_core] = _build(b_core)
    return _PROGRAM_CACHE[b_core]


def make_in_maps(x, weights, b_core=B_CORE, n_cores=N_CORES):
    """Shard x over cores; replicate the (rearranged) parameters."""
    f32 = np.float32
    xr = np.asarray(x, dtype=f32).reshape(-1, 28, 28)
    in_maps = []
    for cidx in range(n_cores):
        xc = xr[cidx * b_core:(cidx + 1) * b_core]  # [b_core, 28, 28]
        xwin = np.zeros((12, 128, 2 * b_core), E4)
        for w in range(6):
            for h in range(2):
                t = w * 2 + h
                win = xc[:, 4 * w:4 * w + 8, 12 * h:12 * h + 16] \
                    .reshape(b_core, 128).astype(f32)
                hi = win.astype(E4)
                lo = (win - hi.astype(f32)).astype(E4)
                lo_f = lo.astype(f32)
                lo_f[:, 127] = 1.0
                lo = lo_f.astype(E4)
                hiT = hi.astype(f32).T.astype(E4)   # [128, b_core]
                loT = lo.astype(f32).T.astype(E4)
                for blk in range(b_core // 128):
                    p, hb = blk // 2, blk % 2
                    base = p * 512 + hb * 256
                    xwin[t, :, base:base + 128] = \
                        hiT[:, blk * 128:(blk + 1) * 128]
                    xwin[t, :, base + 128:base + 256] = \
                        loT[:, blk * 128:(blk + 1) * 128]
        m = {"xw": xwin}
        m.update(weights)
        in_maps.append(m)
    return in_maps


def kernel(**inputs):
    x = np.asarray(inputs["x"], dtype=np.float32)
    weights = _prep_weights(
        np.asarray(inputs["mask_w"], np.float32),
        np.asarray(inputs["conv1_w"], np.float32),
        np.asarray(inputs["conv1_b"], np.float32),
        np.asarray(inputs["conv2_w"], np.float32),
        np.asarray(inputs["conv2_b"], np.float32),
        np.asarray(inputs["fc1_w"], np.float32),
        np.asarray(inputs["fc1_b"], np.float32),
        np.asarray(inputs["fc2_w"], np.float32),
        np.asarray(inputs["fc2_b"], np.float32),
    )
    nc = _get_program(B_CORE)
    in_maps = make_in_maps(x, weights)
    res = run_bass_kernel_spmd(nc, in_maps, list(range(N_CORES)))
    out = np.concatenate([res.results[c]["y"] for c in range(N_CORES)],
                         axis=0)
    return np.ascontiguousarray(out.astype(np.float32))


if __name__ == "__main__":
    rng = np.random.default_rng(0)
    ins = {
        "x": rng.standard_normal((B_TOTAL, 1, 28, 28), dtype=np.float32),
        "mask_w": rng.standard_normal((28, 28), dtype=np.float32) * 0.1,
        "conv1_w": rng.standard_normal((10, 1, 5, 5), dtype=np.float32) * 0.2,
        "conv1_b": rng.standard_normal((10,), dtype=np.float32) * 0.1,
        "conv2_w": rng.standard_normal((20, 10, 5, 5),
                                       dtype=np.float32) * 0.06,
        "conv2_b": rng.standard_normal((20,), dtype=np.float32) * 0.1,
        "fc1_w": rng.standard_normal((50, 320), dtype=np.float32) * 0.05,
        "fc1_b": rng.standard_normal((50,), dtype=np.float32) * 0.1,
        "fc2_w": rng.standard_normal((10, 50), dtype=np.float32) * 0.14,
        "fc2_b": rng.standard_normal((10,), dtype=np.float32) * 0.1,
    }
    out = kernel(**ins)
    print(out.shape, out.dtype, out[:2])
